# revision 4
# baseline (speedup 1.0000x reference)
"""Trainium2 Bass kernel for DiT focused-linear-attention block (nn_DiT_9259949490457).

Data-parallel over batch: 16 batches -> 8 NeuronCores, 2 batches/core, no collectives.
v3: host-pretransposed xT input; PE stripped to essential GEMM columns (q-GEMM,
kv-GEMM, einsum1/2, proj); norms via fused tensor_tensor_reduce on DVE; depthwise
3x3 conv as 9 shifted tensor_scalar/tensor_tensor taps on DVE; per-head q3 tiles
assembled via a DRAM roundtrip (contiguous-row DMA); proj computed feature-major
so its bias is a per-partition ACT bias; host transposes y back.
"""

import numpy as np
import ml_dtypes

import concourse.bacc as bacc
import concourse.mybir as mybir
import concourse.tile as tile
from concourse import bass_utils

F32 = mybir.dt.float32
BF16 = mybir.dt.bfloat16
ALU = mybir.AluOpType
AF = mybir.ActivationFunctionType
AX = mybir.AxisListType

NCORES = 8
B, N, DIM = 16, 1024, 1152
H, KVH, HD = 12, 4, 96
BL = B // NCORES          # 2 local batches
T = BL * N                # 2048 local tokens
NK = DIM // 128           # 9 feature K-tiles
TT = N // 128             # 8 token tiles per batch
C4 = T // 512             # 4 free-dim chunks of 512 over all local tokens
TAPS = [(dy, dx) for dy in (-1, 0, 1) for dx in (-1, 0, 1)]

_BF = ml_dtypes.bfloat16


def _spanp(b):
    if b % 128 == 0:
        return 128
    if b % 64 == 0:
        return 64
    return 32


def _head_pieces(h):
    """Split head h's 96 feature rows into pieces legal for partition-offset
    access both at the 128-aligned global row (r0) and the within-head row (rr).
    Returns [(j_tile, r0, rr, cnt)]."""
    out = []
    rr = 0
    while rr < 96:
        gr = 96 * h + rr
        j, r0 = divmod(gr, 128)
        cnt = min(96 - rr, 128 - r0, _spanp(r0), _spanp(rr))
        out.append((j, r0, rr, cnt))
        rr += cnt
    return out


def _build_kernel():
    nc = bacc.Bacc("TRN2", target_bir_lowering=False, debug=False,
                   enable_asserts=True, num_devices=NCORES)
    xT_in = nc.dram_tensor("xT", [DIM, T], BF16, kind="ExternalInput").ap()
    wqT_in = nc.dram_tensor("wqT", [DIM, DIM], BF16, kind="ExternalInput").ap()
    wkvT_in = nc.dram_tensor("wkvT", [DIM, 768], BF16, kind="ExternalInput").ap()
    pwT_in = nc.dram_tensor("pwT", [DIM, DIM], BF16, kind="ExternalInput").ap()
    wqb_in = nc.dram_tensor("wqb", [128, NK], F32, kind="ExternalInput").ap()
    kvb_in = nc.dram_tensor("kvb", [1, 768], BF16, kind="ExternalInput").ap()
    kvbbc_in = nc.dram_tensor("kvbbc", [128, 768], BF16, kind="ExternalInput").ap()
    pjb_in = nc.dram_tensor("pjb", [128, NK], F32, kind="ExternalInput").ap()
    dwcw_in = nc.dram_tensor("dwcw", [96, KVH, 9], F32, kind="ExternalInput").ap()
    dwcb_in = nc.dram_tensor("dwcb", [96, KVH], F32, kind="ExternalInput").ap()
    masks_in = nc.dram_tensor("masks", [128, NK, H], BF16, kind="ExternalInput").ap()
    y_out = nc.dram_tensor("y", [DIM, T], F32, kind="ExternalOutput").ap()

    from contextlib import ExitStack
    with tile.TileContext(nc) as tc, ExitStack() as stack:
        cpool = stack.enter_context(tc.tile_pool(name="const", bufs=1))
        dpool = stack.enter_context(tc.tile_pool(name="dram", bufs=1, space="DRAM"))
        wp = stack.enter_context(tc.tile_pool(name="work", bufs=1))
        pmm = stack.enter_context(tc.tile_pool(name="pmm", bufs=1, space="PSUM"))
        pa = stack.enter_context(tc.tile_pool(name="pa", bufs=3, space="PSUM"))

        # ---- consts (Pool/SWDGE path, parallel with HWDGE x loads below) ----
        WqT = [cpool.tile([128, DIM], BF16, name=f"WqT{k}") for k in range(NK)]
        WkvT = [cpool.tile([128, 768], BF16, name=f"WkvT{k}") for k in range(NK)]
        PWT = [cpool.tile([128, DIM], BF16, name=f"PWT{k}") for k in range(NK)]
        wqb = cpool.tile([128, NK], F32, name="wqb")
        kvb = cpool.tile([1, 768], BF16, name="kvb")
        kvbbc = cpool.tile([128, 768], BF16, name="kvbbc")
        pjb = cpool.tile([128, NK], F32, name="pjb")
        dwcw = cpool.tile([96, KVH, 9], F32, name="dwcw")
        dwcb = cpool.tile([96, KVH], F32, name="dwcb")
        masks = cpool.tile([128, NK, H], BF16, name="masks")
        ones_r = cpool.tile([1, 128], BF16, name="ones_r")
        ones_c = cpool.tile([128, 1], BF16, name="ones_c")
        nc.vector.memset(ones_r[:], 1.0)
        nc.vector.memset(ones_c[:], 1.0)

        xT = [cpool.tile([128, T], BF16, name=f"xT{k}") for k in range(NK)]
        # interleave: xT[k] via sync/HWDGE, WqT[k] via gpsimd/SWDGE so the
        # first GEMM1 k-steps unblock as early as possible.
        for k in range(NK):
            nc.sync.dma_start(out=xT[k][:], in_=xT_in[128 * k:128 * (k + 1), :])
            nc.gpsimd.dma_start(out=WqT[k][:], in_=wqT_in[128 * k:128 * (k + 1), :])
        nc.gpsimd.dma_start(out=wqb[:], in_=wqb_in[:])
        for k in range(NK):
            nc.gpsimd.dma_start(out=WkvT[k][:], in_=wkvT_in[128 * k:128 * (k + 1), :])
        nc.gpsimd.dma_start(out=kvb[:], in_=kvb_in[:])
        nc.gpsimd.dma_start(out=kvbbc[:], in_=kvbbc_in[:])
        nc.gpsimd.dma_start(out=masks[:], in_=masks_in[:])
        nc.gpsimd.dma_start(out=dwcw[:], in_=dwcw_in[:])
        nc.gpsimd.dma_start(out=dwcb[:], in_=dwcb_in[:])
        for k in range(NK):
            nc.gpsimd.dma_start(out=PWT[k][:], in_=pwT_in[128 * k:128 * (k + 1), :])
        nc.gpsimd.dma_start(out=pjb[:], in_=pjb_in[:])

        vpad = dpool.tile([BL, N, KVH, 128], BF16, name="vpad")
        q3d = dpool.tile([BL, DIM, N], BF16, name="q3d")

        # accs: col = (j, c4) for q, (g, t) for k
        acc1q = wp.tile([128, NK, C4], F32, name="acc1q", tag="acc1q")
        acc2q = wp.tile([128, NK, C4], F32, name="acc2q", tag="acc2q")
        acc1k = wp.tile([128, KVH, 2 * TT], F32, name="acc1k", tag="acc1k")
        acc2k = wp.tile([128, KVH, 2 * TT], F32, name="acc2k", tag="acc2k")

        # ---------------- phase G1: q GEMM + focus(q) ----------------
        for c4 in range(C4):
            t0 = 512 * c4
            for jg in ((0, 1, 2), (3, 4, 5), (6, 7, 8)):
                pq = {j: pmm.tile([128, 512], F32, name=f"pq{j % 4}",
                                  tag=f"pq{j % 4}") for j in jg}
                for k in range(NK):
                    for j in jg:
                        nc.tensor.matmul(pq[j][:], WqT[k][:, 128 * j:128 * (j + 1)],
                                         xT[k][:, t0:t0 + 512],
                                         start=(k == 0), stop=(k == NK - 1))
                for j in jg:
                    u = wp.tile([128, 512], BF16, name="u", tag="u", bufs=2)
                    nc.scalar.activation(u[:], pq[j][:], AF.Relu, bias=wqb[:, j:j + 1])
                    u2 = wp.tile([128, 512], BF16, name="u2", tag="u2", bufs=2)
                    nc.vector.tensor_tensor_reduce(
                        out=u2[:], in0=u[:], in1=u[:], scale=1.0, scalar=0.0,
                        op0=ALU.mult, op1=ALU.add, accum_out=acc1q[:, j, c4:c4 + 1])
                    q3s = wp.tile([128, 512], BF16, name="q3s", tag="q3s", bufs=3)
                    nc.vector.tensor_mul(q3s[:], u2[:], u[:])
                    junk = wp.tile([128, 512], BF16, name="junk", tag="junk", bufs=2)
                    nc.scalar.activation(junk[:], q3s[:], AF.Square,
                                         accum_out=acc2q[:, j, c4:c4 + 1])
                    b = c4 // 2
                    nc.sync.dma_start(
                        out=q3d[b, 128 * j:128 * (j + 1),
                                512 * (c4 % 2):512 * (c4 % 2 + 1)],
                        in_=q3s[:])

        # ---------------- phase K/V + per-batch tail ----------------
        k3 = [wp.tile([128, 384], BF16, name=f"k3_{t}", tag=f"k3_{t}")
              for t in range(2 * TT)]
        vv = [wp.tile([128, 384], BF16, name=f"v_{t}", tag=f"v_{t}")
              for t in range(2 * TT)]
        kvp = [[wp.tile([96, 96], BF16, name=f"kvp{b}_{h}", tag=f"kvp_{h}", bufs=2)
                for h in range(H)] for b in range(BL)]
        vd = [[wp.tile([96, N], BF16, name=f"vd{b}_{g}", tag=f"vd_{g}", bufs=2)
               for g in range(KVH)] for b in range(BL)]
        gbs = []

        for b in range(BL):
            for t in range(TT * b, TT * (b + 1)):
                t0 = 128 * t
                pk = pmm.tile([128, 512], F32, name="pk", tag=f"pq{t % 2}")
                for k in range(NK):
                    nc.tensor.matmul(pk[:, 0:384], xT[k][:, t0:t0 + 128],
                                     WkvT[k][:, 0:384],
                                     start=(k == 0), stop=False)
                nc.tensor.matmul(pk[:, 0:384], ones_r[:], kvb[:, 0:384],
                                 start=False, stop=True)
                uk = wp.tile([128, 384], BF16, name="uk", tag="uk", bufs=2)
                nc.scalar.activation(uk[:], pk[:, 0:384], AF.Relu)
                k2 = wp.tile([128, 384], BF16, name="k2", tag="k2", bufs=2)
                for g in range(KVH):
                    nc.vector.tensor_tensor_reduce(
                        out=k2[:, 96 * g:96 * (g + 1)],
                        in0=uk[:, 96 * g:96 * (g + 1)],
                        in1=uk[:, 96 * g:96 * (g + 1)],
                        scale=1.0, scalar=0.0, op0=ALU.mult, op1=ALU.add,
                        accum_out=acc1k[:, g, t:t + 1])
                nc.vector.tensor_mul(k3[t][:], k2[:], uk[:])
                junkk = wp.tile([128, 384], BF16, name="junkk", tag="junk", bufs=2)
                for g in range(KVH):
                    nc.vector.tensor_tensor_reduce(
                        out=junkk[:, 96 * g:96 * (g + 1)],
                        in0=k3[t][:, 96 * g:96 * (g + 1)],
                        in1=k3[t][:, 96 * g:96 * (g + 1)],
                        scale=1.0, scalar=0.0, op0=ALU.mult, op1=ALU.add,
                        accum_out=acc2k[:, g, t:t + 1])
            for t in range(TT * b, TT * (b + 1)):
                t0 = 128 * t
                pv = pmm.tile([128, 512], F32, name="pv", tag=f"pq{2 + t % 2}")
                for k in range(NK):
                    nc.tensor.matmul(pv[:, 0:384], xT[k][:, t0:t0 + 128],
                                     WkvT[k][:, 384:768],
                                     start=(k == 0), stop=(k == NK - 1))
                nc.vector.tensor_tensor(out=vv[t][:], in0=pv[:, 0:384],
                                        in1=kvbbc[:, 384:768], op=ALU.add)
                nc.sync.dma_start(
                    out=vpad[b, 128 * (t - TT * b):128 * (t - TT * b + 1), :, 0:96],
                    in_=vv[t][:].rearrange("p (k d) -> p k d", k=KVH))

            # ---- norms -> per-head scale gb (tiny) ----
            sq_rows = []
            for acc in (acc1q, acc2q):
                accs = wp.tile([128, NK], F32, name="accs", tag="accs", bufs=2)
                nc.vector.tensor_add(accs[:], acc[:, :, 2 * b], acc[:, :, 2 * b + 1])
                accsb = wp.tile([128, NK], BF16, name="accsb", tag="accsb", bufs=2)
                nc.vector.tensor_copy(accsb[:], accs[:])
                psn = pa.tile([1, H], F32, name="psn", tag="pa")
                for j in range(NK):
                    nc.tensor.matmul(psn[:], accsb[:, j:j + 1], masks[:, j, :],
                                     start=(j == 0), stop=(j == NK - 1))
                srow = wp.tile([1, H], F32, name="srow", tag="srow", bufs=4)
                nc.vector.tensor_copy(srow[:], psn[:])
                sq_rows.append(srow)
            sk_rows = []
            for acc in (acc1k, acc2k):
                accb = wp.tile([128, KVH * TT], BF16, name="accb", tag="accb", bufs=2)
                nc.vector.tensor_copy(accb[:], acc[:, :, TT * b:TT * (b + 1)])
                psk = pa.tile([1, KVH * TT], F32, name="psk", tag="pa")
                nc.tensor.matmul(psk[:], ones_c[:], accb[:], start=True, stop=True)
                krow = wp.tile([1, KVH * TT], F32, name="krow", tag="krow", bufs=2)
                nc.vector.tensor_copy(krow[:], psk[:])
                kred = wp.tile([1, KVH], F32, name="kred", tag="kred", bufs=2)
                nc.vector.tensor_reduce(kred[:],
                                        krow[:].rearrange("a (k t) -> a k t", k=KVH),
                                        axis=AX.X, op=ALU.add)
                sk_rows.append(kred)

            def _f_row(s1, s2, width, tagp):
                se = wp.tile([1, width], F32, name="se", tag=f"se{tagp}", bufs=2)
                nc.vector.tensor_scalar_add(se[:], s2[:], 1e-30)
                rc = wp.tile([1, width], F32, name="rc", tag=f"rc{tagp}", bufs=2)
                nc.vector.reciprocal(rc[:], se[:])
                rt = wp.tile([1, width], F32, name="rt", tag=f"rt{tagp}", bufs=2)
                nc.vector.tensor_mul(rt[:], s1[:], rc[:])
                fr = wp.tile([1, width], F32, name="fr", tag=f"fr{tagp}", bufs=2)
                nc.scalar.activation(fr[:], rt[:], AF.Sqrt)
                return fr

            fq = _f_row(sq_rows[0], sq_rows[1], H, "q")
            fk = _f_row(sk_rows[0], sk_rows[1], KVH, "k")
            fk12 = wp.tile([1, H], F32, name="fk12", tag="fk12", bufs=2)
            for g in range(3):
                nc.vector.tensor_copy(fk12[:, 4 * g:4 * (g + 1)], fk[:])
            grow = wp.tile([1, H], F32, name="grow", tag="grow", bufs=2)
            nc.vector.tensor_mul(grow[:], fq[:], fk12[:])
            gb = wp.tile([96, H], F32, name="gb", tag="gb", bufs=2)
            nc.gpsimd.partition_broadcast(gb[:], grow[:], channels=96)
            gbs.append(gb)

            # ---- einsum1 + scale ----
            for g in range(KVH):
                pk_t = pa.tile([96, 96], F32, name="pkvt", tag="pa")
                for i, t in enumerate(range(TT * b, TT * (b + 1))):
                    nc.tensor.matmul(pk_t[:], k3[t][:, 96 * g:96 * (g + 1)],
                                     vv[t][:, 96 * g:96 * (g + 1)],
                                     start=(i == 0), stop=(i == TT - 1))
                for h in range(g, H, KVH):
                    nc.vector.tensor_scalar(out=kvp[b][h][:], in0=pk_t[:],
                                            scalar1=gb[:, h:h + 1], scalar2=None,
                                            op0=ALU.mult)

            # ---- dwconv branch: 9 shifted taps on DVE ----
            for g in range(KVH):
                vT = wp.tile([128, N], BF16, name="vTd", tag="vTd", bufs=2)
                nc.sync.dma_start(out=vT[:], in_=vpad[b, :, g, :], transpose=True)
                v3 = vT[:].rearrange("p (y x) -> p y x", y=32)
                o3 = vd[b][g][:].rearrange("p (y x) -> p y x", y=32)
                ti0 = TAPS.index((0, 0))
                nc.vector.tensor_scalar(
                    out=vd[b][g][:], in0=vT[0:96, :],
                    scalar1=dwcw[:, g, ti0:ti0 + 1], scalar2=dwcb[:, g:g + 1],
                    op0=ALU.mult, op1=ALU.add)
                for ti, (dy, dx) in enumerate(TAPS):
                    if (dy, dx) == (0, 0):
                        continue
                    y0, y1 = max(0, -dy), 32 - max(0, dy)
                    x0, x1 = max(0, -dx), 32 - max(0, dx)
                    tmp = wp.tile([96, N], BF16, name="tmp", tag="dtmp", bufs=2)
                    t3 = tmp[:].rearrange("p (y x) -> p y x", y=32)
                    nc.vector.tensor_scalar(
                        out=t3[:, y0:y1, x0:x1],
                        in0=v3[0:96, y0 + dy:y1 + dy, x0 + dx:x1 + dx],
                        scalar1=dwcw[:, g, ti:ti + 1], scalar2=None, op0=ALU.mult)
                    nc.vector.tensor_tensor(
                        out=o3[:, y0:y1, x0:x1], in0=o3[:, y0:y1, x0:x1],
                        in1=t3[:, y0:y1, x0:x1], op=ALU.add)

        # ---------------- per-head q3 tiles via DRAM roundtrip ----------------
        q3h = [[wp.tile([96, N], BF16, name=f"q3h{b}_{h}", tag=f"q3h_{h}", bufs=1)
                for h in range(H)] for b in range(BL)]
        for b in range(BL):
            for h in range(H):
                nc.sync.dma_start(out=q3h[b][h][:], in_=q3d[b, 96 * h:96 * (h + 1), :])

        # ---------------- einsum2 + combine -> OTc, proj ----------------
        OTc = [[wp.tile([128, 512], BF16, name=f"OT_{j}_{c}", tag=f"OT_{j}_{c}",
                        bufs=1) for c in range(2)] for j in range(NK)]

        def emit_e2(b, c2, h):
            g = h % KVH
            pe2 = pa.tile([96, 512], F32, name="pe2", tag="pa")
            nc.tensor.matmul(pe2[:], kvp[b][h][:],
                             q3h[b][h][:, 512 * c2:512 * (c2 + 1)],
                             start=True, stop=True)
            pac = wp.tile([96, 512], BF16, name="pac", tag="pac", bufs=2)
            nc.scalar.copy(pac[:], pe2[:])
            for (j, r0, rr, cnt) in _head_pieces(h):
                nc.vector.tensor_tensor(
                    out=OTc[j][c2][r0:r0 + cnt, :],
                    in0=pac[rr:rr + cnt, :],
                    in1=vd[b][g][rr:rr + cnt, 512 * c2:512 * (c2 + 1)],
                    op=ALU.add)

        def emit_proj(b, c2, jo):
            py = pmm.tile([128, 512], F32, name="py", tag=f"pq{jo % 3}")
            for k in range(NK):
                nc.tensor.matmul(py[:], PWT[k][:, 128 * jo:128 * (jo + 1)],
                                 OTc[k][c2][:], start=(k == 0), stop=(k == NK - 1))
            ysb = wp.tile([128, 512], BF16, name="ysb", tag="ysb", bufs=2)
            nc.scalar.activation(ysb[:], py[:], AF.Identity, bias=pjb[:, jo:jo + 1])
            t0 = 1024 * b + 512 * c2
            nc.gpsimd.dma_start(out=y_out[128 * jo:128 * (jo + 1), t0:t0 + 512],
                                in_=ysb[:])

        # b0 einsum2 (both chunks), then proj b0 c2=0;
        # e2 b1 c2=0 interleaves into proj b0 c2=1 (OTc rings free as proj b0
        # finishes reading each chunk), e2 b1 c2=1 into proj b1 c2=0.
        for c2 in range(2):
            for h in range(H):
                emit_e2(0, c2, h)
        for jo in range(NK):
            emit_proj(0, 0, jo)
        e2q = [(1, 0, h) for h in range(H)]
        for jo in range(NK):
            emit_proj(0, 1, jo)
            if e2q and jo % 2 == 0:
                emit_e2(*e2q.pop(0))
        while e2q:
            emit_e2(*e2q.pop(0))
        e2q = [(1, 1, h) for h in range(H)]
        for jo in range(NK):
            emit_proj(1, 0, jo)
            if e2q and jo % 2 == 0:
                emit_e2(*e2q.pop(0))
        while e2q:
            emit_e2(*e2q.pop(0))
        for jo in range(NK):
            emit_proj(1, 1, jo)

    nc.compile()
    return nc


_NC_CACHE = None


def _get_nc():
    global _NC_CACHE
    if _NC_CACHE is None:
        _NC_CACHE = _build_kernel()
    return _NC_CACHE


def _host_consts(wq_w, wq_b, wkv_w, wkv_b, dwc_w, dwc_b, proj_w, proj_b):
    wqT = np.ascontiguousarray(np.asarray(wq_w, np.float32).T).astype(_BF)
    wkvT = np.ascontiguousarray(np.asarray(wkv_w, np.float32).T).astype(_BF)
    pwT = np.ascontiguousarray(np.asarray(proj_w, np.float32).T).astype(_BF)
    wqb = np.ascontiguousarray(np.asarray(wq_b, np.float32).reshape(NK, 128).T)
    kvb_r = np.asarray(wkv_b, np.float32).reshape(1, 768).astype(_BF)
    kvbbc = np.broadcast_to(kvb_r, (128, 768)).astype(_BF)
    pjb = np.ascontiguousarray(np.asarray(proj_b, np.float32).reshape(NK, 128).T)
    dw = np.asarray(dwc_w, np.float32).reshape(KVH, 96, 9)
    dwcw = np.ascontiguousarray(dw.transpose(1, 0, 2))
    dwcb = np.ascontiguousarray(np.asarray(dwc_b, np.float32).reshape(KVH, 96).T)
    mk = np.zeros((128, NK, H), np.float32)
    for j in range(NK):
        for p in range(128):
            f = 128 * j + p
            mk[p, j, f // 96] = 1.0
    masks = mk.astype(_BF)
    return dict(wqT=wqT, wkvT=wkvT, pwT=pwT, wqb=wqb, kvb=kvb_r, kvbbc=kvbbc,
                pjb=pjb, dwcw=dwcw, dwcb=dwcb, masks=masks)


def kernel(x, wq_w, wq_b, wkv_w, wkv_b, dwc_w, dwc_b, proj_w, proj_b,
           _want_results=False, **_unused):
    nc = _get_nc()
    consts = _host_consts(wq_w, wq_b, wkv_w, wkv_b, dwc_w, dwc_b, proj_w, proj_b)
    x = np.asarray(x, np.float32)
    in_maps = []
    for c in range(NCORES):
        m = dict(consts)
        m["xT"] = np.ascontiguousarray(
            x[BL * c:BL * (c + 1)].reshape(T, DIM).T).astype(_BF)
        in_maps.append(m)
    res = bass_utils.run_bass_kernel_spmd(nc, in_maps, core_ids=list(range(NCORES)))
    y = np.stack([np.ascontiguousarray(res.results[c]["y"].T).reshape(BL, N, DIM)
                  for c in range(NCORES)])
    y = y.reshape(B, N, DIM)
    if _want_results:
        return y, res
    return y


# revision 14
# speedup vs baseline: 1.0299x; 1.0299x over previous
"""Trainium2 Bass kernel for DiT focused-linear-attention block (nn_DiT_9259949490457).

Data-parallel over batch: 16 batches -> 8 NeuronCores, 2 batches/core, no collectives.
v3: host-pretransposed xT input; PE stripped to essential GEMM columns (q-GEMM,
kv-GEMM, einsum1/2, proj); norms via fused tensor_tensor_reduce on DVE; depthwise
3x3 conv as 9 shifted tensor_scalar/tensor_tensor taps on DVE; per-head q3 tiles
assembled via a DRAM roundtrip (contiguous-row DMA); proj computed feature-major
so its bias is a per-partition ACT bias; host transposes y back.
"""

import numpy as np
import ml_dtypes

import concourse.bacc as bacc
import concourse.mybir as mybir
import concourse.tile as tile
from concourse import bass_utils

F32 = mybir.dt.float32
BF16 = mybir.dt.bfloat16
ALU = mybir.AluOpType
AF = mybir.ActivationFunctionType
AX = mybir.AxisListType

NCORES = 8
B, N, DIM = 16, 1024, 1152
H, KVH, HD = 12, 4, 96
BL = B // NCORES          # 2 local batches
T = BL * N                # 2048 local tokens
NK = DIM // 128           # 9 feature K-tiles
TT = N // 128             # 8 token tiles per batch
C4 = T // 512             # 4 free-dim chunks of 512 over all local tokens
TAPS = [(dy, dx) for dy in (-1, 0, 1) for dx in (-1, 0, 1)]

_BF = ml_dtypes.bfloat16


def _spanp(b):
    if b % 128 == 0:
        return 128
    if b % 64 == 0:
        return 64
    return 32


def _head_pieces(h):
    """Split head h's 96 feature rows into pieces legal for partition-offset
    access both at the 128-aligned global row (r0) and the within-head row (rr).
    Returns [(j_tile, r0, rr, cnt)]."""
    out = []
    rr = 0
    while rr < 96:
        gr = 96 * h + rr
        j, r0 = divmod(gr, 128)
        cnt = min(96 - rr, 128 - r0, _spanp(r0), _spanp(rr))
        out.append((j, r0, rr, cnt))
        rr += cnt
    return out


def _build_kernel():
    nc = bacc.Bacc("TRN2", target_bir_lowering=False, debug=False,
                   enable_asserts=True, num_devices=NCORES)
    xT_in = nc.dram_tensor("xT", [DIM, T], BF16, kind="ExternalInput").ap()
    wqT_in = nc.dram_tensor("wqT", [DIM, DIM], BF16, kind="ExternalInput").ap()
    wkvT_in = nc.dram_tensor("wkvT", [DIM, 768], BF16, kind="ExternalInput").ap()
    pwT_in = nc.dram_tensor("pwT", [DIM, DIM], BF16, kind="ExternalInput").ap()
    wqb_in = nc.dram_tensor("wqb", [128, NK], F32, kind="ExternalInput").ap()
    kvb_in = nc.dram_tensor("kvb", [1, 768], BF16, kind="ExternalInput").ap()
    kvbbc_in = nc.dram_tensor("kvbbc", [128, 768], BF16, kind="ExternalInput").ap()
    pjb_in = nc.dram_tensor("pjb", [128, NK], F32, kind="ExternalInput").ap()
    dwcw_in = nc.dram_tensor("dwcw", [96, KVH, 9], F32, kind="ExternalInput").ap()
    dwcb_in = nc.dram_tensor("dwcb", [96, KVH], F32, kind="ExternalInput").ap()
    masks_in = nc.dram_tensor("masks", [128, NK, H], BF16, kind="ExternalInput").ap()
    y_out = nc.dram_tensor("y", [DIM, T], F32, kind="ExternalOutput").ap()

    from contextlib import ExitStack
    with tile.TileContext(nc) as tc, ExitStack() as stack:
        cpool = stack.enter_context(tc.tile_pool(name="const", bufs=1))
        dpool = stack.enter_context(tc.tile_pool(name="dram", bufs=1, space="DRAM"))
        wp = stack.enter_context(tc.tile_pool(name="work", bufs=1))
        pmm = stack.enter_context(tc.tile_pool(name="pmm", bufs=1, space="PSUM"))
        pa = stack.enter_context(tc.tile_pool(name="pa", bufs=2, space="PSUM"))

        # ---- consts (Pool/SWDGE path, parallel with HWDGE x loads below) ----
        WqT = [cpool.tile([128, DIM], BF16, name=f"WqT{k}") for k in range(NK)]
        WkvT = [cpool.tile([128, 768], BF16, name=f"WkvT{k}") for k in range(NK)]
        PWT = [cpool.tile([128, DIM], BF16, name=f"PWT{k}") for k in range(NK)]
        wqb = cpool.tile([128, NK], F32, name="wqb")
        kvb = cpool.tile([1, 768], BF16, name="kvb")
        kvbbc = cpool.tile([128, 768], BF16, name="kvbbc")
        pjb = cpool.tile([128, NK], F32, name="pjb")
        dwcw = cpool.tile([96, KVH, 9], F32, name="dwcw")
        dwcb = cpool.tile([96, KVH], F32, name="dwcb")
        masks = cpool.tile([128, NK, H], BF16, name="masks")
        ones_r = cpool.tile([1, 128], BF16, name="ones_r")
        ones_c = cpool.tile([128, 1], BF16, name="ones_c")
        nc.vector.memset(ones_r[:], 1.0)
        nc.vector.memset(ones_c[:], 1.0)

        xT = [cpool.tile([128, T], BF16, name=f"xT{k}") for k in range(NK)]
        # xT loaded in 512-column chunks (c4-outer) via sync/HWDGE so GEMM1's
        # first chunk unblocks ~4us in; weights go via gpsimd/SWDGE in parallel.
        for c4 in range(C4):
            for k in range(NK):
                nc.sync.dma_start(
                    out=xT[k][:, 512 * c4:512 * (c4 + 1)],
                    in_=xT_in[128 * k:128 * (k + 1), 512 * c4:512 * (c4 + 1)])
        for k in range(NK):
            nc.gpsimd.dma_start(out=WqT[k][:], in_=wqT_in[128 * k:128 * (k + 1), :])
        nc.gpsimd.dma_start(out=wqb[:], in_=wqb_in[:])
        for k in range(NK):
            nc.gpsimd.dma_start(out=WkvT[k][:], in_=wkvT_in[128 * k:128 * (k + 1), :])
        nc.gpsimd.dma_start(out=kvb[:], in_=kvb_in[:])
        nc.gpsimd.dma_start(out=kvbbc[:], in_=kvbbc_in[:])
        nc.gpsimd.dma_start(out=masks[:], in_=masks_in[:])
        nc.gpsimd.dma_start(out=dwcw[:], in_=dwcw_in[:])
        nc.gpsimd.dma_start(out=dwcb[:], in_=dwcb_in[:])
        for k in range(NK):
            nc.gpsimd.dma_start(out=PWT[k][:], in_=pwT_in[128 * k:128 * (k + 1), :])
        nc.gpsimd.dma_start(out=pjb[:], in_=pjb_in[:])

        vpad = dpool.tile([BL, N, KVH, 128], BF16, name="vpad")
        q3d = dpool.tile([BL, DIM, N], BF16, name="q3d")

        # accs: col = (j, c4) for q, (g, t) for k
        acc1q = wp.tile([128, NK, C4], F32, name="acc1q", tag="acc1q")
        acc2q = wp.tile([128, NK, C4], F32, name="acc2q", tag="acc2q")
        acc1k = wp.tile([128, KVH, 2 * TT], F32, name="acc1k", tag="acc1k")
        acc2k = wp.tile([128, KVH, 2 * TT], F32, name="acc2k", tag="acc2k")

        q3h = [[wp.tile([96, N], BF16, name=f"q3h{b}_{h}", tag=f"q3h_{h}", bufs=1)
                for h in range(H)] for b in range(BL)]

        # ---------------- phase G1: q GEMM + focus(q) ----------------
        for c4 in range(C4):
            t0 = 512 * c4
            for jg in ((0, 1, 2), (3, 4, 5), (6, 7, 8)):
                pq = {j: pmm.tile([128, 512], F32, name=f"pq{j % 4}",
                                  tag=f"pq{j % 4}") for j in jg}
                for k in range(NK):
                    for j in jg:
                        nc.tensor.matmul(pq[j][:], WqT[k][:, 128 * j:128 * (j + 1)],
                                         xT[k][:, t0:t0 + 512],
                                         start=(k == 0), stop=(k == NK - 1))
                for j in jg:
                    u = wp.tile([128, 512], BF16, name="u", tag="u", bufs=2)
                    nc.scalar.activation(u[:], pq[j][:], AF.Relu, bias=wqb[:, j:j + 1])
                    u2 = wp.tile([128, 512], BF16, name="u2", tag="u2", bufs=2)
                    nc.vector.tensor_tensor_reduce(
                        out=u2[:], in0=u[:], in1=u[:], scale=1.0, scalar=0.0,
                        op0=ALU.mult, op1=ALU.add, accum_out=acc1q[:, j, c4:c4 + 1])
                    q3s = wp.tile([128, 512], BF16, name="q3s", tag="q3s", bufs=3)
                    nc.gpsimd.tensor_tensor(out=q3s[:], in0=u2[:], in1=u[:],
                                            op=ALU.mult)
                    if (c4 + j) % 2 == 0:
                        junk = wp.tile([128, 512], BF16, name="junk", tag="junk",
                                       bufs=2)
                        nc.scalar.activation(junk[:], q3s[:], AF.Square,
                                             accum_out=acc2q[:, j, c4:c4 + 1])
                    else:
                        junk = wp.tile([128, 512], BF16, name="junk", tag="junk",
                                       bufs=2)
                        nc.vector.tensor_tensor_reduce(
                            out=junk[:], in0=q3s[:], in1=q3s[:], scale=1.0,
                            scalar=0.0, op0=ALU.mult, op1=ALU.add,
                            accum_out=acc2q[:, j, c4:c4 + 1])
                    b = c4 // 2
                    nc.sync.dma_start(
                        out=q3d[b, 128 * j:128 * (j + 1),
                                512 * (c4 % 2):512 * (c4 % 2 + 1)],
                        in_=q3s[:])
            if c4 % 2 == 1:
                # batch c4//2's q3d fully written: fetch per-head tiles now so
                # they are resident long before einsum2 needs them.
                bq = c4 // 2
                for h in range(H):
                    nc.sync.dma_start(out=q3h[bq][h][:],
                                      in_=q3d[bq, 96 * h:96 * (h + 1), :])

        # ---------------- phase K/V + per-batch tail ----------------
        k3 = [wp.tile([128, 384], BF16, name=f"k3_{t}", tag=f"k3_{t}")
              for t in range(2 * TT)]
        vv = [wp.tile([128, 384], BF16, name=f"v_{t}", tag=f"v_{t}")
              for t in range(2 * TT)]
        kvp = [[wp.tile([96, 96], BF16, name=f"kvp{b}_{h}", tag=f"kvp_{h}", bufs=2)
                for h in range(H)] for b in range(BL)]
        vd = [[wp.tile([96, N], BF16, name=f"vd{b}_{g}", tag=f"vd_{g}", bufs=2)
               for g in range(KVH)] for b in range(BL)]
        gbs = []

        def emit_dwc(b):
            for g in range(KVH):
                vT = wp.tile([128, N], BF16, name="vTd", tag="vTd", bufs=2)
                nc.sync.dma_start(out=vT[:], in_=vpad[b, :, g, :], transpose=True)
                v3 = vT[:].rearrange("p (y x) -> p y x", y=32)
                o3 = vd[b][g][:].rearrange("p (y x) -> p y x", y=32)
                ti0 = TAPS.index((0, 0))
                nc.vector.tensor_scalar(
                    out=vd[b][g][:], in0=vT[0:96, :],
                    scalar1=dwcw[:, g, ti0:ti0 + 1], scalar2=dwcb[:, g:g + 1],
                    op0=ALU.mult, op1=ALU.add)
                for ti, (dy, dx) in enumerate(TAPS):
                    if (dy, dx) == (0, 0):
                        continue
                    y0, y1 = max(0, -dy), 32 - max(0, dy)
                    x0, x1 = max(0, -dx), 32 - max(0, dx)
                    tmp = wp.tile([96, N], BF16, name="tmp", tag="dtmp", bufs=2)
                    t3 = tmp[:].rearrange("p (y x) -> p y x", y=32)
                    nc.vector.tensor_scalar(
                        out=t3[:, y0:y1, x0:x1],
                        in0=v3[0:96, y0 + dy:y1 + dy, x0 + dx:x1 + dx],
                        scalar1=dwcw[:, g, ti:ti + 1], scalar2=None, op0=ALU.mult)
                    nc.vector.tensor_tensor(
                        out=o3[:, y0:y1, x0:x1], in0=o3[:, y0:y1, x0:x1],
                        in1=t3[:, y0:y1, x0:x1], op=ALU.add)

        for b in range(BL):
            for t in range(TT * b, TT * (b + 1)):
                t0 = 128 * t
                pk = pmm.tile([128, 512], F32, name="pk", tag=f"pq{t % 2}")
                for k in range(NK):
                    nc.tensor.matmul(pk[:, 0:384], xT[k][:, t0:t0 + 128],
                                     WkvT[k][:, 0:384],
                                     start=(k == 0), stop=False)
                nc.tensor.matmul(pk[:, 0:384], ones_r[:], kvb[:, 0:384],
                                 start=False, stop=True)
                uk = wp.tile([128, 384], BF16, name="uk", tag="uk", bufs=2)
                nc.scalar.activation(uk[:], pk[:, 0:384], AF.Relu)
                k2 = wp.tile([128, 384], BF16, name="k2", tag="k2", bufs=2)
                for g in range(KVH):
                    nc.vector.tensor_tensor_reduce(
                        out=k2[:, 96 * g:96 * (g + 1)],
                        in0=uk[:, 96 * g:96 * (g + 1)],
                        in1=uk[:, 96 * g:96 * (g + 1)],
                        scale=1.0, scalar=0.0, op0=ALU.mult, op1=ALU.add,
                        accum_out=acc1k[:, g, t:t + 1])
                nc.vector.tensor_mul(k3[t][:], k2[:], uk[:])
                junkk = wp.tile([128, 384], BF16, name="junkk", tag="junk", bufs=2)
                for g in range(KVH):
                    nc.vector.tensor_tensor_reduce(
                        out=junkk[:, 96 * g:96 * (g + 1)],
                        in0=k3[t][:, 96 * g:96 * (g + 1)],
                        in1=k3[t][:, 96 * g:96 * (g + 1)],
                        scale=1.0, scalar=0.0, op0=ALU.mult, op1=ALU.add,
                        accum_out=acc2k[:, g, t:t + 1])
            for t in range(TT * b, TT * (b + 1)):
                t0 = 128 * t
                pv = pmm.tile([128, 512], F32, name="pv", tag=f"pq{2 + t % 2}")
                for k in range(NK):
                    nc.tensor.matmul(pv[:, 0:384], xT[k][:, t0:t0 + 128],
                                     WkvT[k][:, 384:768],
                                     start=(k == 0), stop=(k == NK - 1))
                nc.vector.tensor_tensor(out=vv[t][:], in0=pv[:, 0:384],
                                        in1=kvbbc[:, 384:768], op=ALU.add)
                nc.sync.dma_start(
                    out=vpad[b, 128 * (t - TT * b):128 * (t - TT * b + 1), :, 0:96],
                    in_=vv[t][:].rearrange("p (k d) -> p k d", k=KVH))

            # ---- norms -> per-head scale gb (tiny) ----
            sq_rows = []
            for acc in (acc1q, acc2q):
                accs = wp.tile([128, NK], F32, name="accs", tag="accs", bufs=2)
                nc.vector.tensor_add(accs[:], acc[:, :, 2 * b], acc[:, :, 2 * b + 1])
                accsb = wp.tile([128, NK], BF16, name="accsb", tag="accsb", bufs=2)
                nc.vector.tensor_copy(accsb[:], accs[:])
                psn = pa.tile([1, H], F32, name="psn", tag="pa")
                for j in range(NK):
                    nc.tensor.matmul(psn[:], accsb[:, j:j + 1], masks[:, j, :],
                                     start=(j == 0), stop=(j == NK - 1))
                srow = wp.tile([1, H], F32, name="srow", tag="srow", bufs=4)
                nc.vector.tensor_copy(srow[:], psn[:])
                sq_rows.append(srow)
            sk_rows = []
            for acc in (acc1k, acc2k):
                accb = wp.tile([128, KVH * TT], BF16, name="accb", tag="accb", bufs=2)
                nc.vector.tensor_copy(accb[:], acc[:, :, TT * b:TT * (b + 1)])
                psk = pa.tile([1, KVH * TT], F32, name="psk", tag="pa")
                nc.tensor.matmul(psk[:], ones_c[:], accb[:], start=True, stop=True)
                krow = wp.tile([1, KVH * TT], F32, name="krow", tag="krow", bufs=2)
                nc.vector.tensor_copy(krow[:], psk[:])
                kred = wp.tile([1, KVH], F32, name="kred", tag="kred", bufs=2)
                nc.vector.tensor_reduce(kred[:],
                                        krow[:].rearrange("a (k t) -> a k t", k=KVH),
                                        axis=AX.X, op=ALU.add)
                sk_rows.append(kred)

            def _f_row(s1, s2, width, tagp):
                se = wp.tile([1, width], F32, name="se", tag=f"se{tagp}", bufs=2)
                nc.vector.tensor_scalar_add(se[:], s2[:], 1e-30)
                rc = wp.tile([1, width], F32, name="rc", tag=f"rc{tagp}", bufs=2)
                nc.vector.reciprocal(rc[:], se[:])
                rt = wp.tile([1, width], F32, name="rt", tag=f"rt{tagp}", bufs=2)
                nc.vector.tensor_mul(rt[:], s1[:], rc[:])
                fr = wp.tile([1, width], F32, name="fr", tag=f"fr{tagp}", bufs=2)
                nc.scalar.activation(fr[:], rt[:], AF.Sqrt)
                return fr

            fq = _f_row(sq_rows[0], sq_rows[1], H, "q")
            fk = _f_row(sk_rows[0], sk_rows[1], KVH, "k")
            fk12 = wp.tile([1, H], F32, name="fk12", tag="fk12", bufs=2)
            for g in range(3):
                nc.vector.tensor_copy(fk12[:, 4 * g:4 * (g + 1)], fk[:])
            grow = wp.tile([1, H], F32, name="grow", tag="grow", bufs=2)
            nc.vector.tensor_mul(grow[:], fq[:], fk12[:])
            gb = wp.tile([96, H], F32, name="gb", tag="gb", bufs=2)
            nc.gpsimd.partition_broadcast(gb[:], grow[:], channels=96)
            gbs.append(gb)

            # ---- einsum1 + scale ----
            for g in range(KVH):
                pk_t = pa.tile([96, 96], F32, name="pkvt", tag="pa")
                for i, t in enumerate(range(TT * b, TT * (b + 1))):
                    nc.tensor.matmul(pk_t[:], k3[t][:, 96 * g:96 * (g + 1)],
                                     vv[t][:, 96 * g:96 * (g + 1)],
                                     start=(i == 0), stop=(i == TT - 1))
                for h in range(g, H, KVH):
                    nc.vector.tensor_scalar(out=kvp[b][h][:], in0=pk_t[:],
                                            scalar1=gb[:, h:h + 1], scalar2=None,
                                            op0=ALU.mult)

            # ---- dwconv branch: 9 shifted taps on DVE (b=1 deferred so its
            # DVE work doesn't sit ahead of e2-b0's evacuations) ----
            if b == 0:
                emit_dwc(0)

        # ---------------- einsum2 + combine -> OTc, proj ----------------
        OTc = [[wp.tile([128, 512], BF16, name=f"OT_{j}_{c}", tag=f"OT_{j}_{c}",
                        bufs=1) for c in range(2)] for j in range(NK)]

        def emit_e2(b, c2, h):
            g = h % KVH
            pe2 = pa.tile([96, 512], F32, name="pe2", tag="pe2", bufs=2)
            nc.tensor.matmul(pe2[:], kvp[b][h][:],
                             q3h[b][h][:, 512 * c2:512 * (c2 + 1)],
                             start=True, stop=True)
            pac = wp.tile([96, 512], BF16, name="pac", tag="pac", bufs=2)
            nc.scalar.copy(pac[:], pe2[:])
            for (j, r0, rr, cnt) in _head_pieces(h):
                nc.vector.tensor_tensor(
                    out=OTc[j][c2][r0:r0 + cnt, :],
                    in0=pac[rr:rr + cnt, :],
                    in1=vd[b][g][rr:rr + cnt, 512 * c2:512 * (c2 + 1)],
                    op=ALU.add)

        def emit_proj(b, c2, jo):
            py = pmm.tile([128, 512], F32, name="py", tag=f"pq{jo % 3}")
            for k in range(NK):
                nc.tensor.matmul(py[:], PWT[k][:, 128 * jo:128 * (jo + 1)],
                                 OTc[k][c2][:], start=(k == 0), stop=(k == NK - 1))
            ysb = wp.tile([128, 512], BF16, name="ysb", tag="ysb", bufs=2)
            nc.scalar.activation(ysb[:], py[:], AF.Identity, bias=pjb[:, jo:jo + 1])
            t0 = 1024 * b + 512 * c2
            nc.gpsimd.dma_start(out=y_out[128 * jo:128 * (jo + 1), t0:t0 + 512],
                                in_=ysb[:])

        # b0 einsum2 (both chunks), then proj b0 c2=0;
        # e2 b1 c2=0 interleaves into proj b0 c2=1 (OTc rings free as proj b0
        # finishes reading each chunk), e2 b1 c2=1 into proj b1 c2=0.
        for c2 in range(2):
            for h in range(H):
                emit_e2(0, c2, h)
        emit_dwc(1)
        for jo in range(NK):
            emit_proj(0, 0, jo)
        e2q = [(1, 0, h) for h in range(H)]
        for jo in range(NK):
            emit_proj(0, 1, jo)
            for _ in range(2):
                if e2q:
                    emit_e2(*e2q.pop(0))
        while e2q:
            emit_e2(*e2q.pop(0))
        e2q = [(1, 1, h) for h in range(H)]
        for jo in range(NK):
            emit_proj(1, 0, jo)
            for _ in range(2):
                if e2q:
                    emit_e2(*e2q.pop(0))
        while e2q:
            emit_e2(*e2q.pop(0))
        for jo in range(NK):
            emit_proj(1, 1, jo)

    nc.compile()
    return nc


_NC_CACHE = None


def _get_nc():
    global _NC_CACHE
    if _NC_CACHE is None:
        _NC_CACHE = _build_kernel()
    return _NC_CACHE


def _host_consts(wq_w, wq_b, wkv_w, wkv_b, dwc_w, dwc_b, proj_w, proj_b):
    wqT = np.ascontiguousarray(np.asarray(wq_w, np.float32).T).astype(_BF)
    wkvT = np.ascontiguousarray(np.asarray(wkv_w, np.float32).T).astype(_BF)
    pwT = np.ascontiguousarray(np.asarray(proj_w, np.float32).T).astype(_BF)
    wqb = np.ascontiguousarray(np.asarray(wq_b, np.float32).reshape(NK, 128).T)
    kvb_r = np.asarray(wkv_b, np.float32).reshape(1, 768).astype(_BF)
    kvbbc = np.broadcast_to(kvb_r, (128, 768)).astype(_BF)
    pjb = np.ascontiguousarray(np.asarray(proj_b, np.float32).reshape(NK, 128).T)
    dw = np.asarray(dwc_w, np.float32).reshape(KVH, 96, 9)
    dwcw = np.ascontiguousarray(dw.transpose(1, 0, 2))
    dwcb = np.ascontiguousarray(np.asarray(dwc_b, np.float32).reshape(KVH, 96).T)
    mk = np.zeros((128, NK, H), np.float32)
    for j in range(NK):
        for p in range(128):
            f = 128 * j + p
            mk[p, j, f // 96] = 1.0
    masks = mk.astype(_BF)
    return dict(wqT=wqT, wkvT=wkvT, pwT=pwT, wqb=wqb, kvb=kvb_r, kvbbc=kvbbc,
                pjb=pjb, dwcw=dwcw, dwcb=dwcb, masks=masks)


def kernel(x, wq_w, wq_b, wkv_w, wkv_b, dwc_w, dwc_b, proj_w, proj_b,
           _want_results=False, **_unused):
    nc = _get_nc()
    consts = _host_consts(wq_w, wq_b, wkv_w, wkv_b, dwc_w, dwc_b, proj_w, proj_b)
    x = np.asarray(x, np.float32)
    in_maps = []
    for c in range(NCORES):
        m = dict(consts)
        m["xT"] = np.ascontiguousarray(
            x[BL * c:BL * (c + 1)].reshape(T, DIM).T).astype(_BF)
        in_maps.append(m)
    res = bass_utils.run_bass_kernel_spmd(nc, in_maps, core_ids=list(range(NCORES)))
    y = np.stack([np.ascontiguousarray(res.results[c]["y"].T).reshape(BL, N, DIM)
                  for c in range(NCORES)])
    y = y.reshape(B, N, DIM)
    if _want_results:
        return y, res
    return y


# revision 17
# speedup vs baseline: 1.1052x; 1.0731x over previous
"""Trainium2 Bass kernel for DiT focused-linear-attention block (nn_DiT_9259949490457).

Data-parallel over batch: 16 batches -> 8 NeuronCores, 2 batches/core, no collectives.
v3: host-pretransposed xT input; PE stripped to essential GEMM columns (q-GEMM,
kv-GEMM, einsum1/2, proj); norms via fused tensor_tensor_reduce on DVE; depthwise
3x3 conv as 9 shifted tensor_scalar/tensor_tensor taps on DVE; per-head q3 tiles
assembled via a DRAM roundtrip (contiguous-row DMA); proj computed feature-major
so its bias is a per-partition ACT bias; host transposes y back.
"""

import numpy as np
import ml_dtypes

import concourse.bacc as bacc
import concourse.mybir as mybir
import concourse.tile as tile
from concourse import bass_utils

F32 = mybir.dt.float32
BF16 = mybir.dt.bfloat16
ALU = mybir.AluOpType
AF = mybir.ActivationFunctionType
AX = mybir.AxisListType

NCORES = 8
B, N, DIM = 16, 1024, 1152
H, KVH, HD = 12, 4, 96
BL = B // NCORES          # 2 local batches
T = BL * N                # 2048 local tokens
NK = DIM // 128           # 9 feature K-tiles
TT = N // 128             # 8 token tiles per batch
C4 = T // 512             # 4 free-dim chunks of 512 over all local tokens
TAPS = [(dy, dx) for dy in (-1, 0, 1) for dx in (-1, 0, 1)]

_BF = ml_dtypes.bfloat16


def _spanp(b):
    if b % 128 == 0:
        return 128
    if b % 64 == 0:
        return 64
    return 32


def _head_pieces(h):
    """Split head h's 96 feature rows into pieces legal for partition-offset
    access both at the 128-aligned global row (r0) and the within-head row (rr).
    Returns [(j_tile, r0, rr, cnt)]."""
    out = []
    rr = 0
    while rr < 96:
        gr = 96 * h + rr
        j, r0 = divmod(gr, 128)
        cnt = min(96 - rr, 128 - r0, _spanp(r0), _spanp(rr))
        out.append((j, r0, rr, cnt))
        rr += cnt
    return out


def _build_kernel():
    nc = bacc.Bacc("TRN2", target_bir_lowering=False, debug=False,
                   enable_asserts=True, num_devices=NCORES)
    xT_in = nc.dram_tensor("xT", [DIM, T], BF16, kind="ExternalInput").ap()
    wqT_in = nc.dram_tensor("wqT", [DIM, DIM], BF16, kind="ExternalInput").ap()
    wkvT_in = nc.dram_tensor("wkvT", [DIM, 768], BF16, kind="ExternalInput").ap()
    pwT_in = nc.dram_tensor("pwT", [DIM, DIM], BF16, kind="ExternalInput").ap()
    wqb_in = nc.dram_tensor("wqb", [128, NK], F32, kind="ExternalInput").ap()
    kvb_in = nc.dram_tensor("kvb", [1, 768], BF16, kind="ExternalInput").ap()
    kvbbc_in = nc.dram_tensor("kvbbc", [128, 384], BF16, kind="ExternalInput").ap()
    pjb_in = nc.dram_tensor("pjb", [128, NK], F32, kind="ExternalInput").ap()
    dwcw_in = nc.dram_tensor("dwcw", [96, KVH, 9], F32, kind="ExternalInput").ap()
    dwcb_in = nc.dram_tensor("dwcb", [96, KVH], F32, kind="ExternalInput").ap()
    masks_in = nc.dram_tensor("masks", [128, NK, H], BF16, kind="ExternalInput").ap()
    y_out = nc.dram_tensor("y", [DIM, T], F32, kind="ExternalOutput").ap()

    from contextlib import ExitStack
    with tile.TileContext(nc) as tc, ExitStack() as stack:
        cpool = stack.enter_context(tc.tile_pool(name="const", bufs=1))
        dpool = stack.enter_context(tc.tile_pool(name="dram", bufs=1, space="DRAM"))
        wp = stack.enter_context(tc.tile_pool(name="work", bufs=1))
        pmm = stack.enter_context(tc.tile_pool(name="pmm", bufs=1, space="PSUM"))
        pa = stack.enter_context(tc.tile_pool(name="pa", bufs=2, space="PSUM"))

        # ---- consts (Pool/SWDGE path, parallel with HWDGE x loads below) ----
        WqT = [cpool.tile([128, DIM], BF16, name=f"WqT{k}") for k in range(NK)]
        WkvT = [cpool.tile([128, 768], BF16, name=f"WkvT{k}") for k in range(NK)]
        PWT = [cpool.tile([128, DIM], BF16, name=f"PWT{k}") for k in range(NK)]
        wqb = cpool.tile([128, NK], F32, name="wqb")
        kvb = cpool.tile([1, 768], BF16, name="kvb")
        kvbbc = cpool.tile([128, 384], BF16, name="kvbbc")
        pjb = cpool.tile([128, NK], F32, name="pjb")
        dwcw = cpool.tile([96, KVH, 9], F32, name="dwcw")
        dwcb = cpool.tile([96, KVH], F32, name="dwcb")
        masks = cpool.tile([128, NK, H], BF16, name="masks")
        ones_r = cpool.tile([1, 128], BF16, name="ones_r")
        ones_c = cpool.tile([128, 1], BF16, name="ones_c")
        nc.vector.memset(ones_r[:], 1.0)
        nc.vector.memset(ones_c[:], 1.0)

        xT = [cpool.tile([128, T], BF16, name=f"xT{k}") for k in range(NK)]
        # xT loaded in 512-column chunks (c4-outer) via sync/HWDGE so GEMM1's
        # first chunk unblocks ~4us in; weights go via gpsimd/SWDGE in parallel.
        for c4 in range(C4):
            for k in range(NK):
                nc.sync.dma_start(
                    out=xT[k][:, 512 * c4:512 * (c4 + 1)],
                    in_=xT_in[128 * k:128 * (k + 1), 512 * c4:512 * (c4 + 1)])
        for k in range(NK):
            nc.gpsimd.dma_start(out=WqT[k][:], in_=wqT_in[128 * k:128 * (k + 1), :])
        nc.gpsimd.dma_start(out=wqb[:], in_=wqb_in[:])
        for k in range(NK):
            nc.gpsimd.dma_start(out=WkvT[k][:], in_=wkvT_in[128 * k:128 * (k + 1), :])
        nc.gpsimd.dma_start(out=kvb[:], in_=kvb_in[:])
        nc.gpsimd.dma_start(out=kvbbc[:], in_=kvbbc_in[:])
        nc.gpsimd.dma_start(out=masks[:], in_=masks_in[:])
        nc.gpsimd.dma_start(out=dwcw[:], in_=dwcw_in[:])
        nc.gpsimd.dma_start(out=dwcb[:], in_=dwcb_in[:])
        for k in range(NK):
            nc.gpsimd.dma_start(out=PWT[k][:], in_=pwT_in[128 * k:128 * (k + 1), :])
        nc.gpsimd.dma_start(out=pjb[:], in_=pjb_in[:])

        vpad = dpool.tile([BL, N, KVH, 128], BF16, name="vpad")
        q3d = dpool.tile([BL, DIM, N], BF16, name="q3d")

        # accs: col = (j, c4) for q, (g, t) for k
        acc1q = wp.tile([128, NK, C4], F32, name="acc1q", tag="acc1q")
        acc2q = wp.tile([128, NK, C4], F32, name="acc2q", tag="acc2q")
        acc1k = wp.tile([128, KVH, 2 * TT], F32, name="acc1k", tag="acc1k")
        acc2k = wp.tile([128, KVH, 2 * TT], F32, name="acc2k", tag="acc2k")

        q3h = [[wp.tile([96, N], BF16, name=f"q3h{b}_{h}", tag=f"q3h_{h}", bufs=1)
                for h in range(H)] for b in range(BL)]

        # ---------------- phase G1: q GEMM + focus(q) ----------------
        for c4 in range(C4):
            t0 = 512 * c4
            for jg in ((0, 1, 2), (3, 4, 5), (6, 7, 8)):
                pq = {j: pmm.tile([128, 512], F32, name=f"pq{j % 4}",
                                  tag=f"pq{j % 4}") for j in jg}
                for k in range(NK):
                    for j in jg:
                        nc.tensor.matmul(pq[j][:], WqT[k][:, 128 * j:128 * (j + 1)],
                                         xT[k][:, t0:t0 + 512],
                                         start=(k == 0), stop=(k == NK - 1))
                for j in jg:
                    u = wp.tile([128, 512], BF16, name="u", tag="u", bufs=2)
                    nc.scalar.activation(u[:], pq[j][:], AF.Relu, bias=wqb[:, j:j + 1])
                    u2 = wp.tile([128, 512], BF16, name="u2", tag="u2", bufs=2)
                    nc.vector.tensor_tensor_reduce(
                        out=u2[:], in0=u[:], in1=u[:], scale=1.0, scalar=0.0,
                        op0=ALU.mult, op1=ALU.add, accum_out=acc1q[:, j, c4:c4 + 1])
                    q3s = wp.tile([128, 512], BF16, name="q3s", tag="q3s", bufs=2)
                    nc.vector.tensor_mul(q3s[:], u2[:], u[:])
                    junk = wp.tile([128, 512], BF16, name="junk", tag="junk",
                                   bufs=2)
                    if (c4 + j) % 2 == 0:
                        nc.scalar.activation(junk[:], q3s[:], AF.Square,
                                             accum_out=acc2q[:, j, c4:c4 + 1])
                    else:
                        nc.vector.tensor_tensor_reduce(
                            out=junk[:], in0=q3s[:], in1=q3s[:], scale=1.0,
                            scalar=0.0, op0=ALU.mult, op1=ALU.add,
                            accum_out=acc2q[:, j, c4:c4 + 1])
                    b = c4 // 2
                    nc.sync.dma_start(
                        out=q3d[b, 128 * j:128 * (j + 1),
                                512 * (c4 % 2):512 * (c4 % 2 + 1)],
                        in_=q3s[:])
            if c4 % 2 == 1:
                # batch c4//2's q3d fully written: fetch per-head tiles now so
                # they are resident long before einsum2 needs them.
                bq = c4 // 2
                for h in range(H):
                    nc.sync.dma_start(out=q3h[bq][h][:],
                                      in_=q3d[bq, 96 * h:96 * (h + 1), :])

        # ---------------- phase K/V + per-batch tail ----------------
        k3 = [wp.tile([128, 384], BF16, name=f"k3_{t}", tag=f"k3_{t}")
              for t in range(2 * TT)]
        vv = [wp.tile([128, 384], BF16, name=f"v_{t}", tag=f"v_{t}")
              for t in range(2 * TT)]
        kvp = [[wp.tile([96, 96], BF16, name=f"kvp{b}_{h}", tag=f"kvp_{h}", bufs=2)
                for h in range(H)] for b in range(BL)]
        vd = [[wp.tile([96, N], BF16, name=f"vd{b}_{g}", tag=f"vd_{g}", bufs=2)
               for g in range(KVH)] for b in range(BL)]
        gbs = []

        def emit_dwc(b):
            for g in range(KVH):
                vT = wp.tile([128, N], BF16, name="vTd", tag="vTd", bufs=2)
                nc.sync.dma_start(out=vT[:], in_=vpad[b, :, g, :], transpose=True)
                v3 = vT[:].rearrange("p (y x) -> p y x", y=32)
                o3 = vd[b][g][:].rearrange("p (y x) -> p y x", y=32)
                ti0 = TAPS.index((0, 0))
                nc.vector.tensor_scalar(
                    out=vd[b][g][:], in0=vT[0:96, :],
                    scalar1=dwcw[:, g, ti0:ti0 + 1], scalar2=dwcb[:, g:g + 1],
                    op0=ALU.mult, op1=ALU.add)
                for ti, (dy, dx) in enumerate(TAPS):
                    if (dy, dx) == (0, 0):
                        continue
                    y0, y1 = max(0, -dy), 32 - max(0, dy)
                    x0, x1 = max(0, -dx), 32 - max(0, dx)
                    tmp = wp.tile([96, N], BF16, name="tmp", tag="dtmp", bufs=2)
                    t3 = tmp[:].rearrange("p (y x) -> p y x", y=32)
                    nc.vector.tensor_scalar(
                        out=t3[:, y0:y1, x0:x1],
                        in0=v3[0:96, y0 + dy:y1 + dy, x0 + dx:x1 + dx],
                        scalar1=dwcw[:, g, ti:ti + 1], scalar2=None, op0=ALU.mult)
                    nc.vector.tensor_tensor(
                        out=o3[:, y0:y1, x0:x1], in0=o3[:, y0:y1, x0:x1],
                        in1=t3[:, y0:y1, x0:x1], op=ALU.add)

        for b in range(BL):
            for t in range(TT * b, TT * (b + 1)):
                t0 = 128 * t
                pk = pmm.tile([128, 512], F32, name="pk", tag=f"pq{t % 2}")
                for k in range(NK):
                    nc.tensor.matmul(pk[:, 0:384], xT[k][:, t0:t0 + 128],
                                     WkvT[k][:, 0:384],
                                     start=(k == 0), stop=False)
                nc.tensor.matmul(pk[:, 0:384], ones_r[:], kvb[:, 0:384],
                                 start=False, stop=True)
                uk = wp.tile([128, 384], BF16, name="uk", tag="uk", bufs=2)
                nc.scalar.activation(uk[:], pk[:, 0:384], AF.Relu)
                k2 = wp.tile([128, 384], BF16, name="k2", tag="k2", bufs=2)
                for g in range(KVH):
                    nc.vector.tensor_tensor_reduce(
                        out=k2[:, 96 * g:96 * (g + 1)],
                        in0=uk[:, 96 * g:96 * (g + 1)],
                        in1=uk[:, 96 * g:96 * (g + 1)],
                        scale=1.0, scalar=0.0, op0=ALU.mult, op1=ALU.add,
                        accum_out=acc1k[:, g, t:t + 1])
                nc.vector.tensor_mul(k3[t][:], k2[:], uk[:])
                junkk = wp.tile([128, 384], BF16, name="junkk", tag="junk", bufs=2)
                for g in range(KVH):
                    nc.vector.tensor_tensor_reduce(
                        out=junkk[:, 96 * g:96 * (g + 1)],
                        in0=k3[t][:, 96 * g:96 * (g + 1)],
                        in1=k3[t][:, 96 * g:96 * (g + 1)],
                        scale=1.0, scalar=0.0, op0=ALU.mult, op1=ALU.add,
                        accum_out=acc2k[:, g, t:t + 1])
            for t in range(TT * b, TT * (b + 1)):
                t0 = 128 * t
                pv = pmm.tile([128, 512], F32, name="pv", tag=f"pq{2 + t % 2}")
                for k in range(NK):
                    nc.tensor.matmul(pv[:, 0:384], xT[k][:, t0:t0 + 128],
                                     WkvT[k][:, 384:768],
                                     start=(k == 0), stop=(k == NK - 1))
                nc.vector.tensor_tensor(out=vv[t][:], in0=pv[:, 0:384],
                                        in1=kvbbc[:], op=ALU.add)
                nc.sync.dma_start(
                    out=vpad[b, 128 * (t - TT * b):128 * (t - TT * b + 1), :, 0:96],
                    in_=vv[t][:].rearrange("p (k d) -> p k d", k=KVH))

            # ---- norms -> per-head scale gb (tiny) ----
            sq_rows = []
            for acc in (acc1q, acc2q):
                accs = wp.tile([128, NK], F32, name="accs", tag="accs", bufs=2)
                nc.vector.tensor_add(accs[:], acc[:, :, 2 * b], acc[:, :, 2 * b + 1])
                accsb = wp.tile([128, NK], BF16, name="accsb", tag="accsb", bufs=2)
                nc.vector.tensor_copy(accsb[:], accs[:])
                psn = pa.tile([1, H], F32, name="psn", tag="pa")
                for j in range(NK):
                    nc.tensor.matmul(psn[:], accsb[:, j:j + 1], masks[:, j, :],
                                     start=(j == 0), stop=(j == NK - 1))
                srow = wp.tile([1, H], F32, name="srow", tag="srow", bufs=4)
                nc.vector.tensor_copy(srow[:], psn[:])
                sq_rows.append(srow)
            sk_rows = []
            for acc in (acc1k, acc2k):
                accb = wp.tile([128, KVH * TT], BF16, name="accb", tag="accb", bufs=2)
                nc.vector.tensor_copy(accb[:], acc[:, :, TT * b:TT * (b + 1)])
                psk = pa.tile([1, KVH * TT], F32, name="psk", tag="pa")
                nc.tensor.matmul(psk[:], ones_c[:], accb[:], start=True, stop=True)
                krow = wp.tile([1, KVH * TT], F32, name="krow", tag="krow", bufs=2)
                nc.vector.tensor_copy(krow[:], psk[:])
                kred = wp.tile([1, KVH], F32, name="kred", tag="kred", bufs=2)
                nc.vector.tensor_reduce(kred[:],
                                        krow[:].rearrange("a (k t) -> a k t", k=KVH),
                                        axis=AX.X, op=ALU.add)
                sk_rows.append(kred)

            def _f_row(s1, s2, width, tagp):
                se = wp.tile([1, width], F32, name="se", tag=f"se{tagp}", bufs=2)
                nc.vector.tensor_scalar_add(se[:], s2[:], 1e-30)
                rc = wp.tile([1, width], F32, name="rc", tag=f"rc{tagp}", bufs=2)
                nc.vector.reciprocal(rc[:], se[:])
                rt = wp.tile([1, width], F32, name="rt", tag=f"rt{tagp}", bufs=2)
                nc.vector.tensor_mul(rt[:], s1[:], rc[:])
                fr = wp.tile([1, width], F32, name="fr", tag=f"fr{tagp}", bufs=2)
                nc.scalar.activation(fr[:], rt[:], AF.Sqrt)
                return fr

            fq = _f_row(sq_rows[0], sq_rows[1], H, "q")
            fk = _f_row(sk_rows[0], sk_rows[1], KVH, "k")
            fk12 = wp.tile([1, H], F32, name="fk12", tag="fk12", bufs=2)
            for g in range(3):
                nc.vector.tensor_copy(fk12[:, 4 * g:4 * (g + 1)], fk[:])
            grow = wp.tile([1, H], F32, name="grow", tag="grow", bufs=2)
            nc.vector.tensor_mul(grow[:], fq[:], fk12[:])
            gb = wp.tile([96, H], F32, name="gb", tag="gb", bufs=2)
            nc.gpsimd.partition_broadcast(gb[:], grow[:], channels=96)
            gbs.append(gb)

            # ---- einsum1 + scale ----
            for g in range(KVH):
                pk_t = pa.tile([96, 96], F32, name="pkvt", tag="pa")
                for i, t in enumerate(range(TT * b, TT * (b + 1))):
                    nc.tensor.matmul(pk_t[:], k3[t][:, 96 * g:96 * (g + 1)],
                                     vv[t][:, 96 * g:96 * (g + 1)],
                                     start=(i == 0), stop=(i == TT - 1))
                for h in range(g, H, KVH):
                    nc.vector.tensor_scalar(out=kvp[b][h][:], in0=pk_t[:],
                                            scalar1=gb[:, h:h + 1], scalar2=None,
                                            op0=ALU.mult)

            # ---- dwconv branch: 9 shifted taps on DVE (b=1 deferred so its
            # DVE work doesn't sit ahead of e2-b0's evacuations) ----
            if b == 0:
                emit_dwc(0)

        # ---------------- einsum2 + combine -> OTc, proj ----------------
        OTc = [[wp.tile([128, 512], BF16, name=f"OT_{j}_{c}", tag=f"OT_{j}_{c}",
                        bufs=1) for c in range(2)] for j in range(NK)]

        def emit_e2(b, c2, h):
            g = h % KVH
            pe2 = pa.tile([96, 512], F32, name="pe2", tag="pe2", bufs=2)
            nc.tensor.matmul(pe2[:], kvp[b][h][:],
                             q3h[b][h][:, 512 * c2:512 * (c2 + 1)],
                             start=True, stop=True)
            pac = wp.tile([96, 512], BF16, name="pac", tag="pac", bufs=2)
            nc.scalar.copy(pac[:], pe2[:])
            for (j, r0, rr, cnt) in _head_pieces(h):
                nc.vector.tensor_tensor(
                    out=OTc[j][c2][r0:r0 + cnt, :],
                    in0=pac[rr:rr + cnt, :],
                    in1=vd[b][g][rr:rr + cnt, 512 * c2:512 * (c2 + 1)],
                    op=ALU.add)

        def emit_proj(b, c2, jo):
            py = pmm.tile([128, 512], F32, name="py", tag=f"pq{jo % 3}")
            for k in range(NK):
                nc.tensor.matmul(py[:], PWT[k][:, 128 * jo:128 * (jo + 1)],
                                 OTc[k][c2][:], start=(k == 0), stop=(k == NK - 1))
            ysb = wp.tile([128, 512], F32, name="ysb", tag="ysb", bufs=2)
            nc.scalar.activation(ysb[:], py[:], AF.Identity, bias=pjb[:, jo:jo + 1])
            t0 = 1024 * b + 512 * c2
            nc.sync.dma_start(out=y_out[128 * jo:128 * (jo + 1), t0:t0 + 512],
                              in_=ysb[:])

        # b0 einsum2 (both chunks), then proj b0 c2=0;
        # e2 b1 c2=0 interleaves into proj b0 c2=1 (OTc rings free as proj b0
        # finishes reading each chunk), e2 b1 c2=1 into proj b1 c2=0.
        for c2 in range(2):
            for h in range(H):
                emit_e2(0, c2, h)
        emit_dwc(1)
        for jo in range(NK):
            emit_proj(0, 0, jo)
        e2q = [(1, 0, h) for h in range(H)]
        for jo in range(NK):
            emit_proj(0, 1, jo)
            for _ in range(2):
                if e2q:
                    emit_e2(*e2q.pop(0))
        while e2q:
            emit_e2(*e2q.pop(0))
        e2q = [(1, 1, h) for h in range(H)]
        for jo in range(NK):
            emit_proj(1, 0, jo)
            for _ in range(2):
                if e2q:
                    emit_e2(*e2q.pop(0))
        while e2q:
            emit_e2(*e2q.pop(0))
        for jo in range(NK):
            emit_proj(1, 1, jo)

    nc.compile()
    return nc


_NC_CACHE = None


def _get_nc():
    global _NC_CACHE
    if _NC_CACHE is None:
        _NC_CACHE = _build_kernel()
    return _NC_CACHE


def _host_consts(wq_w, wq_b, wkv_w, wkv_b, dwc_w, dwc_b, proj_w, proj_b):
    wqT = np.ascontiguousarray(np.asarray(wq_w, np.float32).T).astype(_BF)
    wkvT = np.ascontiguousarray(np.asarray(wkv_w, np.float32).T).astype(_BF)
    pwT = np.ascontiguousarray(np.asarray(proj_w, np.float32).T).astype(_BF)
    wqb = np.ascontiguousarray(np.asarray(wq_b, np.float32).reshape(NK, 128).T)
    kvb_r = np.asarray(wkv_b, np.float32).reshape(1, 768).astype(_BF)
    kvbbc = np.broadcast_to(kvb_r[:, 384:768], (128, 384)).astype(_BF)
    pjb = np.ascontiguousarray(np.asarray(proj_b, np.float32).reshape(NK, 128).T)
    dw = np.asarray(dwc_w, np.float32).reshape(KVH, 96, 9)
    dwcw = np.ascontiguousarray(dw.transpose(1, 0, 2))
    dwcb = np.ascontiguousarray(np.asarray(dwc_b, np.float32).reshape(KVH, 96).T)
    mk = np.zeros((128, NK, H), np.float32)
    for j in range(NK):
        for p in range(128):
            f = 128 * j + p
            mk[p, j, f // 96] = 1.0
    masks = mk.astype(_BF)
    return dict(wqT=wqT, wkvT=wkvT, pwT=pwT, wqb=wqb, kvb=kvb_r, kvbbc=kvbbc,
                pjb=pjb, dwcw=dwcw, dwcb=dwcb, masks=masks)


def kernel(x, wq_w, wq_b, wkv_w, wkv_b, dwc_w, dwc_b, proj_w, proj_b,
           _want_results=False, **_unused):
    nc = _get_nc()
    consts = _host_consts(wq_w, wq_b, wkv_w, wkv_b, dwc_w, dwc_b, proj_w, proj_b)
    x = np.asarray(x, np.float32)
    in_maps = []
    for c in range(NCORES):
        m = dict(consts)
        m["xT"] = np.ascontiguousarray(
            x[BL * c:BL * (c + 1)].reshape(T, DIM).T).astype(_BF)
        in_maps.append(m)
    res = bass_utils.run_bass_kernel_spmd(nc, in_maps, core_ids=list(range(NCORES)))
    y = np.stack([np.ascontiguousarray(res.results[c]["y"].T).reshape(BL, N, DIM)
                  for c in range(NCORES)])
    y = y.reshape(B, N, DIM)
    if _want_results:
        return y, res
    return y
